# revision 3
# baseline (speedup 1.0000x reference)
"""Trainium2 Bass kernel for ColumnParallelLinearWithTopping.

Computes  y[t] = x[t] @ (W_base.T + DeltaW[j] + A[j] @ B[j]),  j = weight_indices[t]

Strategy (8-core tensor parallel over the output dim, 512 cols/core):
  * Host: stable-argsort tokens by adapter id, pack x rows grouped by
    adapter (each group padded to a multiple of 128 rows), and ship x
    TRANSPOSED ([D_IN, T_pad]) in bf16 so the device never transposes
    activations.  The effective weights
        W_eff[a] = W_base.T + DeltaW[a] + A[a] @ B[a]
    are combined on host (rank-16 update + elementwise adds, ~1.5% of
    total FLOPs) and shipped column-sharded in bf16.
  * Device (per core, SPMD): pure grouped GEMM in bf16 (full-rate PE),
    fp32 PSUM accumulation.  For each adapter a, stream its token
    blocks 6 at a time:
        psum_y[b][tok,512] += xT[k, tokens_b].T @ W_eff[a][k]
  * Host: concatenate per-core column shards and undo the permutation.
"""
from contextlib import ExitStack

import ml_dtypes
import numpy as np

import concourse.bass as bass
import concourse.mybir as mybir
import concourse.tile as tile
from concourse import bacc
from concourse.bass_utils import run_bass_kernel_spmd

T, D_IN, D_OUT = 8192, 4096, 4096
N_ADAPT, RANK = 8, 16
N_CORES = 8
P = 128
SHARD = D_OUT // N_CORES          # 512 output cols per core
KT = D_IN // P                    # 32 contraction tiles
GRP = 6                           # token blocks per GEMM group
F32 = mybir.dt.float32
BF16 = mybir.dt.bfloat16
NP_BF16 = ml_dtypes.bfloat16

_build_cache: dict = {}


def _build(nb: tuple, nvalid: tuple):
    """Build + compile the SPMD program for per-adapter block counts nb."""
    t_pad = sum(nb) * P
    nc = bacc.Bacc("TRN2", target_bir_lowering=False, debug=False)
    xt = nc.dram_tensor("xt", [D_IN, t_pad], BF16, kind="ExternalInput").ap()
    weff = nc.dram_tensor("weff", [N_ADAPT, KT // 4, P, 4 * SHARD], BF16,
                          kind="ExternalInput").ap()
    y = nc.dram_tensor("y", [t_pad, SHARD], F32, kind="ExternalOutput").ap()

    with tile.TileContext(nc) as tc, ExitStack() as ctx:
        w_pool = ctx.enter_context(tc.tile_pool(name="wp", bufs=16))
        xt_pool = ctx.enter_context(tc.tile_pool(name="xtp", bufs=8))
        y_pool = ctx.enter_context(tc.tile_pool(name="yo", bufs=4))
        psum_y = ctx.enter_context(tc.tile_pool(name="psum_y", bufs=1, space="PSUM"))

        blk_base = 0
        for a in range(N_ADAPT):
            if nb[a] == 0:
                continue
            # ---- full W_eff[a] column shard into SBUF: 8 DMAs of [128, 4*512]
            wt4 = []
            for k4 in range(KT // 4):
                wt = w_pool.tile([P, 4, SHARD], BF16, name="wt")
                nc.scalar.dma_start(
                    wt, weff[a, k4].rearrange("p (i n) -> p i n", i=4))
                wt4.append(wt)
            wtiles = [wt4[k // 4][:, k % 4, :] for k in range(KT)]

            # ---- grouped GEMM: up to GRP token blocks at a time ----
            blk = 0
            while blk < nb[a]:
                g = min(GRP, nb[a] - blk)
                tok0 = (blk_base + blk) * P
                W = g * P
                psums = [psum_y.tile([P, SHARD], F32, name=f"py{b}",
                                     tag=f"py{b}", bufs=1)
                         for b in range(g)]
                for k in range(KT):
                    xt_sb = xt_pool.tile([P, GRP * P], BF16, name="xt_sb")
                    nc.sync.dma_start(
                        xt_sb[:, :W], xt[k * P:(k + 1) * P, tok0:tok0 + W])
                    for b in range(g):
                        nc.tensor.matmul(
                            psums[b],
                            xt_sb[:, b * P:(b + 1) * P],
                            wtiles[k],
                            start=(k == 0), stop=(k == KT - 1),
                        )
                for b in range(g):
                    y_sb = y_pool.tile([P, SHARD], F32, name="y_sb")
                    nc.vector.tensor_copy(y_sb, psums[b])
                    nc.scalar.dma_start(
                        y[tok0 + b * P:tok0 + (b + 1) * P, :], y_sb)
                blk += g
            blk_base += nb[a]

    nc.compile()
    return nc, t_pad


def kernel(x, weight_indices, W_base, A_buffer, B_buffer, DeltaW):
    x = np.asarray(x, dtype=np.float32)
    idx = np.asarray(weight_indices).astype(np.int64)
    W_base = np.asarray(W_base, dtype=np.float32)
    A_buffer = np.asarray(A_buffer, dtype=np.float32)
    B_buffer = np.asarray(B_buffer, dtype=np.float32)
    DeltaW = np.asarray(DeltaW, dtype=np.float32)

    order = np.argsort(idx, kind="stable")
    counts = np.bincount(idx, minlength=N_ADAPT)
    nb = tuple(int(-(-c // P)) for c in counts)
    t_pad = sum(nb) * P

    nvalid = tuple(int(c) for c in counts)
    key = (nb, nvalid)
    if key not in _build_cache:
        _build_cache[key] = _build(nb, nvalid)
    nc, _ = _build_cache[key]

    # pack x columns (transposed) grouped by adapter, pad to 128-row blocks
    xT = np.ascontiguousarray(x.T).astype(NP_BF16)  # [D_IN, T] bf16
    xt_packed = np.zeros((D_IN, t_pad), dtype=NP_BF16)
    seg_dst = []          # (dst_row0, count, sorted_token_slice_start)
    cum = np.concatenate([[0], np.cumsum(counts)])
    row0 = 0
    for a in range(N_ADAPT):
        c = int(counts[a])
        if c:
            xt_packed[:, row0:row0 + c] = xT[:, order[cum[a]:cum[a] + c]]
        seg_dst.append((row0, c, int(cum[a])))
        row0 += nb[a] * P

    # W_eff[a] = W_base.T + DeltaW[a] + A[a] @ B[a]   (host, fp32 -> bf16)
    W_eff = DeltaW + W_base.T[None, :, :]
    W_eff += np.einsum("aik,akj->aij", A_buffer, B_buffer, optimize=True)
    W_eff = W_eff.astype(NP_BF16)                    # [A, D_IN, D_OUT]

    in_maps = []
    for c in range(N_CORES):
        sl = slice(c * SHARD, (c + 1) * SHARD)
        in_maps.append({
            "xt": xt_packed,
            "weff": np.ascontiguousarray(
                W_eff[:, :, sl].reshape(N_ADAPT, KT // 4, 4, P, SHARD)
                .transpose(0, 1, 3, 2, 4)).reshape(
                    N_ADAPT, KT // 4, P, 4 * SHARD),
        })

    global _last_in_maps
    _last_in_maps = in_maps
    res = run_bass_kernel_spmd(nc, in_maps, core_ids=list(range(N_CORES)))
    y_packed = np.concatenate(
        [res.results[c]["y"] for c in range(N_CORES)], axis=1)  # [t_pad, D_OUT]

    out = np.empty((T, D_OUT), dtype=np.float32)
    for a in range(N_ADAPT):
        row0, c, s = seg_dst[a]
        if c:
            out[order[s:s + c]] = y_packed[row0:row0 + c]
    return out


# revision 4
# speedup vs baseline: 1.1575x; 1.1575x over previous
"""Trainium2 Bass kernel for ColumnParallelLinearWithTopping.

Computes  y[t] = x[t] @ (W_base.T + DeltaW[j] + A[j] @ B[j]),  j = weight_indices[t]

Strategy (8-core tensor parallel over the output dim, 512 cols/core):
  * Host: stable-argsort tokens by adapter id, pack x rows grouped by
    adapter (each group padded to a multiple of 128 rows), and ship x
    TRANSPOSED ([D_IN, T_pad]) in bf16 so the device never transposes
    activations.  The effective weights
        W_eff[a] = W_base.T + DeltaW[a] + A[a] @ B[a]
    are combined on host (rank-16 update + elementwise adds, ~1.5% of
    total FLOPs) and shipped column-sharded in bf16.
  * Device (per core, SPMD): pure grouped GEMM in bf16 (full-rate PE),
    fp32 PSUM accumulation.  For each adapter a, stream its token
    blocks 6 at a time:
        psum_y[b][tok,512] += xT[k, tokens_b].T @ W_eff[a][k]
  * Host: concatenate per-core column shards and undo the permutation.
"""
from contextlib import ExitStack

import ml_dtypes
import numpy as np

import concourse.bass as bass
import concourse.mybir as mybir
import concourse.tile as tile
from concourse import bacc
from concourse.bass_utils import run_bass_kernel_spmd

T, D_IN, D_OUT = 8192, 4096, 4096
N_ADAPT, RANK = 8, 16
N_CORES = 8
P = 128
SHARD = D_OUT // N_CORES          # 512 output cols per core
KT = D_IN // P                    # 32 contraction tiles
GRP = 6                           # token blocks per GEMM group
F32 = mybir.dt.float32
BF16 = mybir.dt.bfloat16
NP_BF16 = ml_dtypes.bfloat16

_build_cache: dict = {}


def _build(nb: tuple, nvalid: tuple):
    """Build + compile the SPMD program for per-adapter block counts nb."""
    t_pad = sum(nb) * P
    nc = bacc.Bacc("TRN2", target_bir_lowering=False, debug=False)
    xt = nc.dram_tensor("xt", [D_IN, t_pad], BF16, kind="ExternalInput").ap()
    weff = nc.dram_tensor("weff", [N_ADAPT, KT // 4, P, 4 * SHARD], BF16,
                          kind="ExternalInput").ap()
    y = nc.dram_tensor("y", [t_pad, SHARD], F32, kind="ExternalOutput").ap()

    with tile.TileContext(nc) as tc, ExitStack() as ctx:
        w_pool = ctx.enter_context(tc.tile_pool(name="wp", bufs=16))
        xt_pool = ctx.enter_context(tc.tile_pool(name="xtp", bufs=10))
        y_pool = ctx.enter_context(tc.tile_pool(name="yo", bufs=4))
        psum_y = ctx.enter_context(tc.tile_pool(name="psum_y", bufs=1, space="PSUM"))

        xdma = 0
        blk_base = 0
        for a in range(N_ADAPT):
            if nb[a] == 0:
                continue
            # ---- full W_eff[a] column shard into SBUF: 8 DMAs of [128, 4*512]
            wt4 = []
            for k4 in range(KT // 4):
                wt = w_pool.tile([P, 4, SHARD], BF16, name="wt")
                nc.scalar.dma_start(
                    wt, weff[a, k4].rearrange("p (i n) -> p i n", i=4))
                wt4.append(wt)
            wtiles = [wt4[k // 4][:, k % 4, :] for k in range(KT)]

            # ---- grouped GEMM: balanced groups of <= GRP token blocks ----
            ng = -(-nb[a] // GRP)
            base_sz, extra = divmod(nb[a], ng)
            sizes = [base_sz + (1 if i < extra else 0) for i in range(ng)]
            blk = 0
            for g in sizes:
                tok0 = (blk_base + blk) * P
                W = g * P
                psums = [psum_y.tile([P, SHARD], F32, name=f"py{b}",
                                     tag=f"py{b}", bufs=1)
                         for b in range(g)]
                for k4 in range(KT // 4):
                    # one batched DMA per 4 k-tiles, alternating HWDGE queues
                    xt_sb = xt_pool.tile([P, 4, GRP * P], BF16, name="xt_sb")
                    eng = nc.sync if xdma % 2 == 0 else nc.scalar
                    xdma += 1
                    eng.dma_start(
                        xt_sb[:, :, :W],
                        xt[k4 * 4 * P:(k4 + 1) * 4 * P, tok0:tok0 + W]
                        .rearrange("(i p) t -> p i t", p=P))
                    for i in range(4):
                        k = k4 * 4 + i
                        for b in range(g):
                            nc.tensor.matmul(
                                psums[b],
                                xt_sb[:, i, b * P:(b + 1) * P],
                                wtiles[k],
                                start=(k == 0), stop=(k == KT - 1),
                            )
                for b in range(g):
                    y_sb = y_pool.tile([P, SHARD], F32, name="y_sb")
                    nc.vector.tensor_copy(y_sb, psums[b])
                    nc.scalar.dma_start(
                        y[tok0 + b * P:tok0 + (b + 1) * P, :], y_sb)
                blk += g
            blk_base += nb[a]

    nc.compile()
    return nc, t_pad


def kernel(x, weight_indices, W_base, A_buffer, B_buffer, DeltaW):
    x = np.asarray(x, dtype=np.float32)
    idx = np.asarray(weight_indices).astype(np.int64)
    W_base = np.asarray(W_base, dtype=np.float32)
    A_buffer = np.asarray(A_buffer, dtype=np.float32)
    B_buffer = np.asarray(B_buffer, dtype=np.float32)
    DeltaW = np.asarray(DeltaW, dtype=np.float32)

    order = np.argsort(idx, kind="stable")
    counts = np.bincount(idx, minlength=N_ADAPT)
    nb = tuple(int(-(-c // P)) for c in counts)
    t_pad = sum(nb) * P

    nvalid = tuple(int(c) for c in counts)
    key = (nb, nvalid)
    if key not in _build_cache:
        _build_cache[key] = _build(nb, nvalid)
    nc, _ = _build_cache[key]

    # pack x columns (transposed) grouped by adapter, pad to 128-row blocks
    xT = np.ascontiguousarray(x.T).astype(NP_BF16)  # [D_IN, T] bf16
    xt_packed = np.zeros((D_IN, t_pad), dtype=NP_BF16)
    seg_dst = []          # (dst_row0, count, sorted_token_slice_start)
    cum = np.concatenate([[0], np.cumsum(counts)])
    row0 = 0
    for a in range(N_ADAPT):
        c = int(counts[a])
        if c:
            xt_packed[:, row0:row0 + c] = xT[:, order[cum[a]:cum[a] + c]]
        seg_dst.append((row0, c, int(cum[a])))
        row0 += nb[a] * P

    # W_eff[a] = W_base.T + DeltaW[a] + A[a] @ B[a]   (host, fp32 -> bf16)
    W_eff = DeltaW + W_base.T[None, :, :]
    W_eff += np.einsum("aik,akj->aij", A_buffer, B_buffer, optimize=True)
    W_eff = W_eff.astype(NP_BF16)                    # [A, D_IN, D_OUT]

    in_maps = []
    for c in range(N_CORES):
        sl = slice(c * SHARD, (c + 1) * SHARD)
        in_maps.append({
            "xt": xt_packed,
            "weff": np.ascontiguousarray(
                W_eff[:, :, sl].reshape(N_ADAPT, KT // 4, 4, P, SHARD)
                .transpose(0, 1, 3, 2, 4)).reshape(
                    N_ADAPT, KT // 4, P, 4 * SHARD),
        })

    global _last_in_maps
    _last_in_maps = in_maps
    res = run_bass_kernel_spmd(nc, in_maps, core_ids=list(range(N_CORES)))
    y_packed = np.concatenate(
        [res.results[c]["y"] for c in range(N_CORES)], axis=1)  # [t_pad, D_OUT]

    out = np.empty((T, D_OUT), dtype=np.float32)
    for a in range(N_ADAPT):
        row0, c, s = seg_dst[a]
        if c:
            out[order[s:s + c]] = y_packed[row0:row0 + c]
    return out


# revision 5
# speedup vs baseline: 1.2499x; 1.0798x over previous
"""Trainium2 Bass kernel for ColumnParallelLinearWithTopping.

Computes  y[t] = x[t] @ (W_base.T + DeltaW[j] + A[j] @ B[j]),  j = weight_indices[t]

Strategy (8-core tensor parallel over the output dim, 512 cols/core):
  * Host: stable-argsort tokens by adapter id, ship x TRANSPOSED
    ([D_IN, T], bf16, adapter-sorted, NO padding).  The effective weights
        W_eff[a] = W_base.T + DeltaW[a] + A[a] @ B[a]
    are combined on host (rank-16 update + elementwise adds, ~1.5% of
    total FLOPs) and shipped column-sharded in bf16.
  * Device (per core, SPMD): pure GEMM in bf16 (full-rate PE), fp32 PSUM.
    W_eff tiles are the STATIONARY operand; tokens stream as the moving
    free dim in chunks of <=512, so ragged per-adapter token counts cost
    no padding.  Output is produced transposed:
        psum[cc][col 128, tok n] += W_eff[a][k, cc*128:+128].T @ xT[k, chunk]
    accumulated over k = 0..31, for cc = 0..3 column chunks.
  * Host: concatenate per-core column shards ([512, T] each), transpose,
    undo the permutation.
"""
from contextlib import ExitStack

import ml_dtypes
import numpy as np

import concourse.bass as bass
import concourse.mybir as mybir
import concourse.tile as tile
from concourse import bacc
from concourse.bass_utils import run_bass_kernel_spmd

T, D_IN, D_OUT = 8192, 4096, 4096
N_ADAPT, RANK = 8, 16
N_CORES = 8
P = 128
SHARD = D_OUT // N_CORES          # 512 output cols per core
KT = D_IN // P                    # 32 contraction tiles
NC_CHUNK = 512                    # max tokens streamed per matmul
F32 = mybir.dt.float32
BF16 = mybir.dt.bfloat16
NP_BF16 = ml_dtypes.bfloat16

_build_cache: dict = {}


def _chunks(c: int) -> list:
    """Balanced split of c tokens into ceil(c/512) chunks (sizes <= 512)."""
    if c == 0:
        return []
    n = -(-c // NC_CHUNK)
    base, extra = divmod(c, n)
    return [base + (1 if i < extra else 0) for i in range(n)]


def _build(nvalid: tuple):
    """Build + compile the SPMD program for per-adapter token counts."""
    nc = bacc.Bacc("TRN2", target_bir_lowering=False, debug=False)
    xt = nc.dram_tensor("xt", [D_IN, T], BF16, kind="ExternalInput").ap()
    weff = nc.dram_tensor("weff", [N_ADAPT, KT // 4, P, 4 * SHARD], BF16,
                          kind="ExternalInput").ap()
    yt = nc.dram_tensor("yt", [SHARD, T], F32, kind="ExternalOutput").ap()

    with tile.TileContext(nc) as tc, ExitStack() as ctx:
        w_pool = ctx.enter_context(tc.tile_pool(name="wp", bufs=16))
        xt_pool = ctx.enter_context(tc.tile_pool(name="xtp", bufs=10))
        y_pool = ctx.enter_context(tc.tile_pool(name="yo", bufs=8))
        psum_y = ctx.enter_context(tc.tile_pool(name="psum_y", bufs=1, space="PSUM"))

        gci = 0                     # global chunk counter (PSUM parity, queues)
        tok0 = 0
        for a in range(N_ADAPT):
            if nvalid[a] == 0:
                continue
            first_adapter = tok0 == 0
            # ---- full W_eff[a] column shard into SBUF: 8 DMAs of [128, 4*512]
            wt4 = []
            for k4 in range(KT // 4):
                wt = w_pool.tile([P, 4, SHARD], BF16, name="wt")
                nc.scalar.dma_start(
                    wt, weff[a, k4].rearrange("p (i n) -> p i n", i=4))
                wt4.append(wt)

            for ci, n in enumerate(_chunks(nvalid[a])):
                par = gci % 2
                psums = [psum_y.tile([P, NC_CHUNK], F32, name=f"ps{cc}_{par}",
                                     tag=f"ps{cc}_{par}", bufs=1)
                         for cc in range(4)]
                for k4 in range(KT // 4):
                    # batched x DMA: 4 k-tiles of the token chunk
                    xt_sb = xt_pool.tile([P, 4, NC_CHUNK], BF16, name="xt_sb")
                    if first_adapter and ci == 0:
                        eng = nc.sync        # keep scalar free for W stream
                    else:
                        eng = nc.sync if (gci * 8 + k4) % 2 == 0 else nc.scalar
                    eng.dma_start(
                        xt_sb[:, :, :n],
                        xt[k4 * 4 * P:(k4 + 1) * 4 * P, tok0:tok0 + n]
                        .rearrange("(i p) t -> p i t", p=P))
                    for i in range(4):
                        k = k4 * 4 + i
                        for cc in range(4):
                            nc.tensor.matmul(
                                psums[cc][:, :n],
                                wt4[k4][:, i, cc * P:(cc + 1) * P],
                                xt_sb[:, i, :n],
                                start=(k == 0), stop=(k == KT - 1),
                            )
                for cc in range(4):
                    y_sb = y_pool.tile([P, NC_CHUNK], F32, name="y_sb")
                    nc.vector.tensor_copy(y_sb[:, :n], psums[cc][:, :n])
                    nc.scalar.dma_start(
                        yt[cc * P:(cc + 1) * P, tok0:tok0 + n], y_sb[:, :n])
                tok0 += n
                gci += 1

    nc.compile()
    return nc


def kernel(x, weight_indices, W_base, A_buffer, B_buffer, DeltaW):
    x = np.asarray(x, dtype=np.float32)
    idx = np.asarray(weight_indices).astype(np.int64)
    W_base = np.asarray(W_base, dtype=np.float32)
    A_buffer = np.asarray(A_buffer, dtype=np.float32)
    B_buffer = np.asarray(B_buffer, dtype=np.float32)
    DeltaW = np.asarray(DeltaW, dtype=np.float32)

    order = np.argsort(idx, kind="stable")
    counts = np.bincount(idx, minlength=N_ADAPT)
    nvalid = tuple(int(c) for c in counts)
    if nvalid not in _build_cache:
        _build_cache[nvalid] = _build(nvalid)
    nc = _build_cache[nvalid]

    # x columns (transposed) in adapter-sorted order, bf16, no padding
    xT = np.ascontiguousarray(x.T).astype(NP_BF16)   # [D_IN, T] bf16
    xt_packed = np.ascontiguousarray(xT[:, order])

    # W_eff[a] = W_base.T + DeltaW[a] + A[a] @ B[a]   (host, fp32 -> bf16)
    W_eff = DeltaW + W_base.T[None, :, :]
    W_eff += np.einsum("aik,akj->aij", A_buffer, B_buffer, optimize=True)
    W_eff = W_eff.astype(NP_BF16)                    # [A, D_IN, D_OUT]

    in_maps = []
    for c in range(N_CORES):
        sl = slice(c * SHARD, (c + 1) * SHARD)
        in_maps.append({
            "xt": xt_packed,
            "weff": np.ascontiguousarray(
                W_eff[:, :, sl].reshape(N_ADAPT, KT // 4, 4, P, SHARD)
                .transpose(0, 1, 3, 2, 4)).reshape(
                    N_ADAPT, KT // 4, P, 4 * SHARD),
        })

    global _last_in_maps
    _last_in_maps = in_maps
    res = run_bass_kernel_spmd(nc, in_maps, core_ids=list(range(N_CORES)))
    yt_full = np.concatenate(
        [res.results[c]["yt"] for c in range(N_CORES)], axis=0)  # [D_OUT, T]

    out = np.empty((T, D_OUT), dtype=np.float32)
    out[order] = yt_full.T
    return out
